# revision 19
# baseline (speedup 1.0000x reference)
"""Mixer (token-mix + channel-mix MLP) kernel for 8 TRN2 NeuronCores.

Strategy (expert-style parallel over the group axes):
  Phase 1 (C-sharded): core m owns channels Cm=[32m,32m+32). LN1 is folded into
  the per-channel PE transpose (augmented matmul whose moving operand is
  [diag(rstd); -mu*rstd] next to an identity, yielding both (x-mu)*rstd and x
  transposed in one matmul); g1/be1 are folded into the fc1 weights/bias on
  the host. Token-mix fc1+fc2 run in bf16 with per-channel [128,128]
  stationary blocks in transposed layout ([feature, batch]); accumulation is
  fp32 in PSUM. The residual u = x + tok is accumulated in fp32 into an SBUF
  staging buffer; LN2 stats come from an fp32 gram matmul on uT with a ones
  column appended ([sum u*u | sum u]).
  AllToAll: the u staging buffer + LN2 stats ship so core k owns patches
  Nk=[32k,32k+32) with all 256 channels.
  Phase 2 (N-sharded): yn = (u-mu2)*rstd2 is materialized (bf16) on the
  receive side from the shipped stats (g2/be2 folded into cw1/bias on host),
  then the channel-mix fc1+fc2 and final fp32 residual run per patch; output
  stays in [patch, channel, batch] layout and the host transposes it back.
"""
import sys
import numpy as np

sys.path.insert(0, "/opt/trn_rl_repo")

import ml_dtypes
import concourse.bass as bass
import concourse.bacc as bacc
import concourse.tile as tile
from concourse import mybir
from concourse.bass_utils import run_bass_kernel_spmd

F32 = mybir.dt.float32
BF16 = mybir.dt.bfloat16
NCORE = 8
B, C, N = 64, 256, 256
CL = C // NCORE   # 32 local channels (phase 1)
NL = N // NCORE   # 32 local patches (phase 2)
EPS = 1e-5
GELU = mybir.ActivationFunctionType.Gelu

U_ELEMS = NL * CL * B            # 65536 per a2a block
ST_ELEMS = B * CL * 2            # 4096 stats elems per a2a block
BLK_ELEMS = U_ELEMS + ST_ELEMS   # 69632


def build_program(gelu_func=GELU, mmdt=BF16, skip_b2=False, skip_bc2=False):
    nc = bacc.Bacc("TRN2", target_bir_lowering=False, debug=False,
                   enable_asserts=True, num_devices=NCORE)

    x_in = nc.dram_tensor("x_sh", [B, CL, N], F32, kind="ExternalInput")
    wt_in = nc.dram_tensor("wt", [CL, 128, 4, N], mmdt, kind="ExternalInput")
    ct_in = nc.dram_tensor("ct", [NL, 128, 4, C], mmdt, kind="ExternalInput")
    b1t_in = nc.dram_tensor("b1t", [128, 2, CL], F32, kind="ExternalInput")
    b2t_in = nc.dram_tensor("b2t", [128, 2, CL], F32, kind="ExternalInput")
    bc1t_in = nc.dram_tensor("bc1t", [128, 2, NL], F32, kind="ExternalInput")
    bc2t_in = nc.dram_tensor("bc2t", [128, 2, NL], F32, kind="ExternalInput")
    id64_in = nc.dram_tensor("id64", [64, 64], F32, kind="ExternalInput")
    idx_in = nc.dram_tensor("idx65", [65, 64], F32, kind="ExternalInput")

    ybuf = nc.dram_tensor("ybuf", [NL, C, B], F32, kind="ExternalOutput")

    with tile.TileContext(nc) as tc:
        with tc.tile_pool(name="const", bufs=1) as const, \
             tc.tile_pool(name="wpool", bufs=3) as wpool, \
             tc.tile_pool(name="act", bufs=3) as act, \
             tc.tile_pool(name="small", bufs=4) as small, \
             tc.tile_pool(name="dram", bufs=1, space="DRAM") as dram, \
             tc.tile_pool(name="ps", bufs=2, space="PSUM") as ps:

            send_t = dram.tile([NCORE, BLK_ELEMS], F32)
            recv_t = dram.tile([NCORE, BLK_ELEMS], F32)
            send_u = send_t[:, 0:U_ELEMS].rearrange(
                "j (nl c b) -> j nl c b", nl=NL, c=CL, b=B)
            send_st = send_t[:, U_ELEMS:BLK_ELEMS].rearrange(
                "j (b c t) -> j b c t", b=B, c=CL, t=2)

            def recv_u_view(j):
                # [32c_loc, 32nl, 64b] view of src-core j's u (layout [nl,c,b])
                return bass.AP(tensor=recv_t.tensor,
                               offset=j * BLK_ELEMS,
                               ap=[[B, CL], [CL * B, NL], [1, B]])

            def recv_st_view(j, comp):
                # [32c, 64b] view of src-core j's stats (layout [b, c, t])
                return bass.AP(tensor=recv_t.tensor,
                               offset=j * BLK_ELEMS + U_ELEMS + comp,
                               ap=[[2, CL], [CL * 2, B]])

            # ---- constants / persistent tiles ----
            x_aug = const.tile([65, CL, N], F32)      # rows 0-63 = x[b], row 64 = 1
            nc.sync.dma_start(out=x_aug[0:64, :, :], in_=x_in[:])
            nc.vector.memset(x_aug[64:65, :, :], 1.0)
            id64 = const.tile([64, 64], F32)
            nc.sync.dma_start(out=id64[:], in_=id64_in[:])
            idx65 = const.tile([65, 64], F32)
            nc.sync.dma_start(out=idx65[:], in_=idx_in[:])
            b1t = const.tile([128, 2, CL], F32)
            nc.sync.dma_start(out=b1t[:], in_=b1t_in[:])
            b2t = const.tile([128, 2, CL], F32)
            nc.sync.dma_start(out=b2t[:], in_=b2t_in[:])
            bc1t = const.tile([128, 2, NL], F32)
            nc.sync.dma_start(out=bc1t[:], in_=bc1t_in[:])
            bc2t = const.tile([128, 2, NL], F32)
            nc.sync.dma_start(out=bc2t[:], in_=bc2t_in[:])
            eps64 = const.tile([64, 1], F32)
            nc.vector.memset(eps64[:], EPS)

            mv_all = const.tile([64, CL, 2], F32)     # LN1 mean/var
            rstd1_all = const.tile([64, CL], F32)
            nmr1_all = const.tile([64, CL], F32)      # -mu1*rstd1
            mu2_all = const.tile([64, CL], F32)
            var2_all = const.tile([64, CL], F32)
            rstd2_all = const.tile([64, CL], F32)
            nmr2_all = const.tile([64, CL], F32)      # -mu2*rstd2
            # per-channel moving operand: [diag(rstd1); -mu*rstd] | [I64; 0]
            movings2 = const.tile([65, CL, 2, 64], F32)
            u_stage = const.tile([128, 2, CL, 64], F32)   # uT for all channels
            ua = const.tile([128, 2, 65], F32)            # [u | 1] gram rhs
            nc.vector.memset(ua[:, :, 64:65], 1.0)
            sum_all = const.tile([64, CL], F32)           # sum_n u
            esq_all = const.tile([64, CL], F32)           # E[u^2]
            dg = const.tile([64, 64], F32)                # gram*mask scratch

            # ---- phase 1a: LN1 stats for all channels ----
            for c in range(CL):
                st6 = small.tile([64, 6], F32)
                nc.vector.bn_stats(out=st6[:], in_=x_aug[0:64, c, :])
                nc.vector.bn_aggr(out=mv_all[:, c, :], in_=st6[:])
            nc.scalar.activation(out=rstd1_all[:], in_=mv_all[:, :, 1],
                                 func=mybir.ActivationFunctionType.Sqrt,
                                 bias=eps64[:], scale=1.0)
            nc.vector.reciprocal(out=rstd1_all[:], in_=rstd1_all[:])
            nc.vector.tensor_mul(out=nmr1_all[:], in0=mv_all[:, :, 0],
                                 in1=rstd1_all[:])
            nc.vector.tensor_scalar_mul(out=nmr1_all[:], in0=nmr1_all[:],
                                        scalar1=-1.0)
            for c in range(CL):
                nc.vector.tensor_scalar_mul(out=movings2[0:64, c, 0, :],
                                            in0=id64[:],
                                            scalar1=rstd1_all[:, c:c + 1])
                nc.vector.tensor_copy(out=movings2[:, c, 1, :], in_=idx65[:])
            # transpose nmr1 [64b, 32c] -> [32c, 64b] into movings2 row 64
            vt_in = const.tile([64, 32], F32)
            nc.vector.tensor_copy(out=vt_in[:], in_=nmr1_all[:])
            vt_out = const.tile([64, 32], F32)
            nc.vector.transpose(out=vt_out[:], in_=vt_in[:])
            nc.sync.dma_start(out=movings2[64:65, :, 0, 0:32], in_=vt_out[0:32, :])
            nc.sync.dma_start(out=movings2[64:65, :, 0, 32:64], in_=vt_out[32:64, :])

            # ---- phase 1b: token mixing per channel ----
            for c in range(CL):
                w12 = wpool.tile([128, 4, N], mmdt, tag="w")
                nc.scalar.dma_start(out=w12[:], in_=wt_in[c])

                # zx[:, blk, 0:64] = (x-mu)*rstd transposed; [64:128] = x.T
                zx = ps.tile([128, 2, 128], F32, tag="zx")
                for blk in range(2):
                    nc.tensor.matmul(
                        zx[:, blk, :],
                        x_aug[:, c, blk * 128:(blk + 1) * 128],
                        movings2[:, c, :, :].rearrange("p a b -> p (a b)"),
                        start=True, stop=True)
                z_sb = act.tile([128, 2, 64], mmdt, tag="z")
                nc.vector.tensor_copy(out=z_sb[:], in_=zx[:, :, 0:64])

                hpre = ps.tile([128, 2, 64], F32, tag="hpre")
                for mb in range(2):
                    for nb in range(2):
                        nc.tensor.matmul(
                            hpre[:, mb, :],
                            w12[:, nb, mb * 128:(mb + 1) * 128],
                            z_sb[:, nb, :],
                            start=(nb == 0), stop=(nb == 1))
                hs = act.tile([128, 2, 64], mmdt, tag="h")
                for mb in range(2):
                    nc.scalar.activation(out=hs[:, mb, :], in_=hpre[:, mb, :],
                                         func=gelu_func, bias=b1t[:, mb, c:c + 1])

                tokp = ps.tile([128, 2, 64], F32, tag="tokp")
                for kb in range(2):
                    for mb in range(2):
                        nc.tensor.matmul(
                            tokp[:, kb, :],
                            w12[:, 2 + mb, kb * 128:(kb + 1) * 128],
                            hs[:, mb, :],
                            start=(mb == 0), stop=(mb == 1))
                # u = x.T + tok (+ tb2): DVE may read only one PSUM input/op
                if skip_b2:
                    nc.vector.tensor_copy(out=u_stage[:, :, c, :],
                                          in_=zx[:, :, 64:128])
                else:
                    for kb in range(2):
                        nc.vector.tensor_scalar(
                            out=u_stage[:, kb, c, :], in0=zx[:, kb, 64:128],
                            scalar1=b2t[:, kb, c:c + 1], scalar2=None,
                            op0=mybir.AluOpType.add)
                nc.vector.tensor_add(out=u_stage[:, :, c, :],
                                     in0=u_stage[:, :, c, :], in1=tokp[:])

                # LN2 stats: gram of uT with a ones column ([sum uu | sum u])
                nc.vector.tensor_copy(out=ua[:, :, 0:64], in_=u_stage[:, :, c, :])
                gs = ps.tile([64, 65], F32, tag="gs")
                for blk in range(2):
                    nc.tensor.matmul(gs[:], u_stage[:, blk, c, :], ua[:, blk, :],
                                     start=(blk == 0), stop=(blk == 1))
                nc.vector.tensor_mul(out=dg[:], in0=gs[:, 0:64], in1=id64[:])
                nc.vector.reduce_sum(out=esq_all[:, c:c + 1], in_=dg[:],
                                     axis=mybir.AxisListType.X)
                nc.vector.tensor_copy(out=sum_all[:, c:c + 1], in_=gs[:, 64:65])

            # ---- phase 1c: LN2 rstd batch + ship u and stats ----
            nc.vector.tensor_scalar(
                out=mu2_all[:], in0=sum_all[:], scalar1=1.0 / N, scalar2=None,
                op0=mybir.AluOpType.mult)
            nc.vector.tensor_scalar(
                out=esq_all[:], in0=esq_all[:], scalar1=1.0 / N, scalar2=None,
                op0=mybir.AluOpType.mult)
            nc.vector.tensor_mul(out=var2_all[:], in0=mu2_all[:],
                                 in1=mu2_all[:])
            nc.vector.tensor_sub(out=var2_all[:], in0=esq_all[:],
                                 in1=var2_all[:])
            nc.scalar.activation(out=rstd2_all[:], in_=var2_all[:],
                                 func=mybir.ActivationFunctionType.Sqrt,
                                 bias=eps64[:], scale=1.0)
            nc.vector.reciprocal(out=rstd2_all[:], in_=rstd2_all[:])
            nc.vector.tensor_mul(out=nmr2_all[:], in0=mu2_all[:],
                                 in1=rstd2_all[:])
            nc.vector.tensor_scalar_mul(out=nmr2_all[:], in0=nmr2_all[:],
                                        scalar1=-1.0)
            stats_il = const.tile([64, CL, 2], F32)
            nc.vector.tensor_copy(out=stats_il[:, :, 0], in_=nmr2_all[:])
            nc.vector.tensor_copy(out=stats_il[:, :, 1], in_=rstd2_all[:])
            for blk in range(2):
                for jr in range(4):
                    j = blk * 4 + jr
                    nc.sync.dma_start(
                        out=send_u[j],
                        in_=u_stage[jr * 32:(jr + 1) * 32, blk, :, :])
            for j in range(NCORE):
                nc.sync.dma_start(out=send_st[j], in_=stats_il[:])

            # ---- collective ----
            nc.gpsimd.collective_compute(
                "AllToAll",
                mybir.AluOpType.bypass,
                replica_groups=[list(range(NCORE))],
                ins=[send_t.opt()],
                outs=[recv_t.opt()],
            )

            # ---- phase 2a: normalization maps + u staging [c, b] ----
            nm_map = const.tile([128, 2, 64], F32)
            rs_map = const.tile([128, 2, 64], F32)
            recv_stage = const.tile([128, 2, NL, 64], F32)
            for ch in range(2):
                for jr in range(4):
                    j = 4 * ch + jr
                    nc.sync.dma_start(
                        out=nm_map[jr * 32:(jr + 1) * 32, ch, :],
                        in_=recv_st_view(j, 0))
                    nc.sync.dma_start(
                        out=rs_map[jr * 32:(jr + 1) * 32, ch, :],
                        in_=recv_st_view(j, 1))
                    nc.sync.dma_start(
                        out=recv_stage[jr * 32:(jr + 1) * 32, ch, :, :],
                        in_=recv_u_view(j))
            y_stage = const.tile([128, 2, NL, 64], F32)

            # ---- phase 2b: channel mixing per patch ----
            for nl in range(NL):
                c12 = wpool.tile([128, 4, C], mmdt, tag="w")
                nc.scalar.dma_start(out=c12[:], in_=ct_in[nl])

                u_n = recv_stage[:, :, nl, :]
                t2 = act.tile([128, 2, 64], F32, tag="t2")
                nc.vector.tensor_mul(out=t2[:], in0=u_n, in1=rs_map[:])
                yn = act.tile([128, 2, 64], mmdt, tag="yn")
                nc.vector.tensor_add(out=yn[:], in0=t2[:], in1=nm_map[:])

                h2p = ps.tile([128, 2, 64], F32, tag="hpre")
                for ob in range(2):
                    for cb in range(2):
                        nc.tensor.matmul(
                            h2p[:, ob, :],
                            c12[:, cb, ob * 128:(ob + 1) * 128],
                            yn[:, cb, :],
                            start=(cb == 0), stop=(cb == 1))
                h2s = act.tile([128, 2, 64], mmdt, tag="h")
                for ob in range(2):
                    nc.scalar.activation(out=h2s[:, ob, :], in_=h2p[:, ob, :],
                                         func=gelu_func,
                                         bias=bc1t[:, ob, nl:nl + 1])

                chp = ps.tile([128, 2, 64], F32, tag="tokp")
                for kb in range(2):
                    for ob in range(2):
                        nc.tensor.matmul(
                            chp[:, kb, :],
                            c12[:, 2 + ob, kb * 128:(kb + 1) * 128],
                            h2s[:, ob, :],
                            start=(ob == 0), stop=(ob == 1))
                if skip_bc2:
                    nc.vector.tensor_add(out=y_stage[:, :, nl, :], in0=chp[:],
                                         in1=u_n)
                else:
                    t3 = act.tile([128, 2, 64], F32, tag="t3")
                    for kb in range(2):
                        nc.vector.tensor_scalar(
                            out=t3[:, kb, :], in0=chp[:, kb, :],
                            scalar1=bc2t[:, kb, nl:nl + 1], scalar2=None,
                            op0=mybir.AluOpType.add)
                    nc.vector.tensor_add(out=y_stage[:, :, nl, :], in0=t3[:],
                                         in1=u_n)

            # ---- output: ybuf[nl, c, b] from y_stage[(k_lo), kb, nl, b] ----
            for kb in range(2):
                out_ap = bass.AP(tensor=ybuf,
                                 offset=kb * 128 * B,
                                 ap=[[B, 128], [C * B, NL], [1, B]])
                nc.sync.dma_start(out=out_ap, in_=y_stage[:, kb, :, :])

    nc.finalize()
    return nc


def prep_inputs(x, g1, be1, g2, be2, tw1, tb1, tw2, tb2, cw1, cb1, cw2, cb2,
                mmdt_np=ml_dtypes.bfloat16):
    """Host-side sharding + weight folding. Returns in_maps for the 8 cores."""
    f = np.float32
    bf = mmdt_np
    x = np.asarray(x, f)
    g1, be1, g2, be2 = (np.asarray(a, f) for a in (g1, be1, g2, be2))
    tw1, tb1, tw2, tb2 = (np.asarray(a, f) for a in (tw1, tb1, tw2, tb2))
    cw1, cb1, cw2, cb2 = (np.asarray(a, f) for a in (cw1, cb1, cw2, cb2))

    # token-mix fc1: fold g1 into weights, be1 into bias; lhsT layout [c, n, m]
    w1t = (tw1 * g1[None, None, :]).transpose(0, 2, 1)            # [C, N, N]
    bias1 = tb1 + np.einsum('n,cmn->cm', be1, tw1)                # [C, M]
    w2t = tw2.transpose(0, 2, 1)                                  # [c, m, k]
    t1r = w1t.reshape(C, 2, 128, N)
    t2r = w2t.reshape(C, 2, 128, N)
    wt = np.ascontiguousarray(
        np.stack([t1r[:, 0], t1r[:, 1], t2r[:, 0], t2r[:, 1]],
                 axis=2)).astype(bf)                              # [C, 128, 4, N]

    # channel-mix fc1: fold g2 (per-patch scalar) into cw1, be2 into bias
    c1t = (cw1 * g2[:, None, None]).transpose(0, 2, 1)            # [N, C, C]
    biasc1 = cb1 + be2[:, None] * cw1.sum(axis=2)                 # [N, O]
    c2t = cw2.transpose(0, 2, 1)                                  # [n, o, k]
    c1r = c1t.reshape(N, 2, 128, C)
    c2r = c2t.reshape(N, 2, 128, C)
    ct = np.ascontiguousarray(
        np.stack([c1r[:, 0], c1r[:, 1], c2r[:, 0], c2r[:, 1]],
                 axis=2)).astype(bf)                              # [N, 128, 4, C]

    id64 = np.eye(64, dtype=f)
    idx65 = np.vstack([np.eye(64, dtype=f), np.zeros((1, 64), f)])

    def fold_bias(bm):   # [G, 256] -> [128, 2, G]
        return np.ascontiguousarray(bm.T.reshape(2, 128, -1).transpose(1, 0, 2))

    in_maps = []
    for m in range(NCORE):
        cs = slice(m * CL, (m + 1) * CL)
        ns = slice(m * NL, (m + 1) * NL)
        in_maps.append({
            "x_sh": np.ascontiguousarray(x[:, cs, :]),
            "wt": np.ascontiguousarray(wt[cs]),
            "ct": np.ascontiguousarray(ct[ns]),
            "b1t": fold_bias(bias1[cs]),
            "b2t": fold_bias(tb2[cs]),
            "bc1t": fold_bias(biasc1[ns]),
            "bc2t": fold_bias(cb2[ns]),
            "id64": id64,
            "idx65": idx65,
        })
    return in_maps


def assemble_output(results):
    """results: list of per-core dicts with 'ybuf' [NL, C, B] -> y [B, C, N]."""
    y = np.empty((B, C, N), np.float32)
    for k in range(NCORE):
        y[:, :, k * NL:(k + 1) * NL] = results[k]["ybuf"].transpose(2, 1, 0)
    return y


_PROGRAMS = {}


def get_program(skip_b2, skip_bc2):
    key = (skip_b2, skip_bc2)
    if key not in _PROGRAMS:
        _PROGRAMS[key] = build_program(skip_b2=skip_b2, skip_bc2=skip_bc2)
    return _PROGRAMS[key]


def kernel(**inputs):
    skip_b2 = not np.any(np.asarray(inputs["tb2"]))
    skip_bc2 = not np.any(np.asarray(inputs["cb2"]))
    prog = get_program(skip_b2, skip_bc2)
    in_maps = prep_inputs(**inputs)
    res = run_bass_kernel_spmd(prog, in_maps, list(range(NCORE)))
    return assemble_output(res.results)


if __name__ == "__main__":
    from scipy.special import erf

    rng = np.random.RandomState(0)
    s = 0.02
    inputs = dict(
        x=rng.randn(B, C, N).astype(np.float32),
        g1=np.ones(N, np.float32), be1=np.zeros(N, np.float32),
        g2=np.ones(N, np.float32), be2=np.zeros(N, np.float32),
        tw1=(rng.randn(C, N, N) * s).astype(np.float32),
        tb1=np.zeros((C, N), np.float32),
        tw2=(rng.randn(C, N, N) * s).astype(np.float32),
        tb2=np.zeros((C, N), np.float32),
        cw1=(rng.randn(N, C, C) * s).astype(np.float32),
        cb1=np.zeros((N, C), np.float32),
        cw2=(rng.randn(N, C, C) * s).astype(np.float32),
        cb2=np.zeros((N, C), np.float32),
    )

    def np_ref(x, g1, be1, g2, be2, tw1, tb1, tw2, tb2, cw1, cb1, cw2, cb2):
        def ln(z, g, b):
            mu = z.mean(-1, keepdims=True)
            var = z.var(-1, keepdims=True)
            return (z - mu) / np.sqrt(var + EPS) * g + b
        def gelu(v):
            return v * 0.5 * (1 + erf(v / np.sqrt(2.0)))
        xn = ln(x, g1, be1)
        h = gelu(np.einsum('bcn,cmn->bcm', xn, tw1) + tb1[None])
        tok = np.einsum('bcm,ckm->bck', h, tw2) + tb2[None]
        x = x + tok
        yn = ln(x, g2, be2)
        h2 = gelu(np.einsum('bcn,noc->bon', yn, cw1) + cb1.T[None])
        ch = np.einsum('bon,nko->bkn', h2, cw2) + cb2.T[None]
        return x + ch

    exp = np_ref(**{k: v.astype(np.float64) for k, v in inputs.items()})
    got = kernel(**inputs)
    err = np.abs(got - exp)
    rel = err.max() / np.abs(exp).max()
    print(f"abs err: {err.max():.3e}  rel(absmax): {rel:.3e}")
